# revision 7
# baseline (speedup 1.0000x reference)
"""Trainium2 Bass kernel for CrossMerge3D.

Input ys: [B=2, S=12, C=96, 32, 32, 32] f32. For each (b, c):
  out = (m0 + perm_j(m1) + perm_k(m2)) / 12
where, with the 12 scans split into 3 groups of 4, each group combines as
  m_g = s0 + s1 + flip(s2 + s3)   (flip over the flattened 32^3 volume)
and group 1's volume is stored as (j,k,i), group 2's as (k,i,j); perm_j /
perm_k bring them back to (i,j,k).

Sharding: 8 cores = batch (2) x channel quarters (4) -> 24 channels/core.
No cross-core communication.

Per-core layout: 4 channels x 32 leading-spatial -> 128 SBUF partitions,
1024-wide free dim. Loads are plain mergeable scan-pair DMAs (1 MiB, fast
HWDGE path) split across the SP and ACT rings.

v1 rework (DVE was the bottleneck at ~83% busy, DMA only 43% MBU):
- pair-sums are plain adds split DVE (fwd) / GpSimd (rev), writing bf16
  sum tiles; the flip's free-dim reversal and group 2's (i,j)->(j,i)
  free permute ride the matmul moving-operand APs instead of costing
  DVE passes.
- each merge m_g accumulates in PSUM via two bf16 matmuls (1 cyc/row,
  4x faster than the 2-pass fp32 path): ps_g = (I/12)@f_g + (J/12)@rev(r_g)
  with J the 32-block exchange; the /12 rides the stationaries for free.
- DVE keeps only the two 32x32 stream transposes (PSUM -> SBUF) and the
  two final adds; final phase is software-pipelined one group behind the
  loads/adds/matmuls so DVE order never stalls the stream.
"""

import numpy as np

_B, _S, _C, _D = 2, 12, 96, 32
_NCORE = 8
_CL = _C // 4          # 24 channels per core
_G = _CL // 4          # 6 macro tiles of 4 channels (128 partitions)
_FREE = _D * _D        # 1024

_nc = None


def _build_program():
    from concourse import bacc, tile, mybir

    f32 = mybir.dt.float32
    bf16 = mybir.dt.bfloat16
    nc = bacc.Bacc(
        "TRN2", target_bir_lowering=False, debug=False, num_devices=_NCORE
    )
    ys = nc.dram_tensor("ys", [_S, _CL, _D, _D, _D], f32, kind="ExternalInput")
    out = nc.dram_tensor("out", [_CL, _D, _D, _D], f32, kind="ExternalOutput")
    ysa = ys.ap()
    outa = out.ap()

    with tile.TileContext(nc) as tc:
        with (
            tc.tile_pool(name="const", bufs=1) as cst,
            tc.tile_pool(name="io", bufs=3) as iop,
            tc.tile_pool(name="tmp", bufs=2) as tmp,
            tc.tile_pool(name="ps", bufs=1, space="PSUM") as ps,
        ):
            # stationaries (bf16 -> 1 cyc/row matmuls), final /12 folded in:
            #   ident12 = I/12; jblk12 = 32-block anti-diagonal / 12
            ident = cst.tile([128, 128], bf16, tag="ident", name="ident")
            nc.gpsimd.memset(ident[:], 1.0 / 12.0)
            nc.gpsimd.affine_select(
                out=ident[:], in_=ident[:],
                compare_op=mybir.AluOpType.is_equal, fill=0.0,
                base=0, pattern=[[1, 128]], channel_multiplier=-1,
            )
            jblk = cst.tile([128, 128], bf16, tag="jblk", name="jblk")
            nc.gpsimd.memset(jblk[:], 1.0 / 12.0)
            for b in range(4):
                nc.gpsimd.affine_select(
                    out=jblk[32 * b:32 * b + 32, :],
                    in_=jblk[32 * b:32 * b + 32, :],
                    compare_op=mybir.AluOpType.is_equal, fill=0.0,
                    base=-(32 * b + 31), pattern=[[1, 128]],
                    channel_multiplier=1,
                )
            I12 = ident[:]
            J12 = jblk[:]

            finish_prev = None

            for g in range(_G):
                cs = slice(4 * g, 4 * (g + 1))

                def load_pair(s, tag, eng):
                    t = iop.tile([128, 2 * _FREE], f32, tag=tag, name=tag)
                    src = ysa[s:s + 2, cs].rearrange(
                        "s c i j k -> (c i) s (j k)"
                    )
                    dst = t[:].rearrange("p (s f) -> p s f", s=2)
                    eng.dma_start(out=dst, in_=src)
                    return t

                pa = load_pair(0, "pa", nc.sync)
                pr = load_pair(2, "pr", nc.scalar)
                qa = load_pair(4, "qa", nc.sync)
                qr = load_pair(6, "qr", nc.scalar)
                ra = load_pair(8, "ra", nc.sync)
                rr = load_pair(10, "rr", nc.scalar)

                def pair_sum(t, tag, eng):
                    # f32 + f32 -> bf16 sum tile (matmul moving operand)
                    s = iop.tile([128, _FREE], bf16, tag=tag, name=tag,
                                 bufs=2)
                    eng.tensor_add(s[:], t[:, 0:_FREE], t[:, _FREE:2 * _FREE])
                    return s[:]

                fA = pair_sum(pa, "fA", nc.vector)
                rA = pair_sum(pr, "rA", nc.gpsimd)
                fB = pair_sum(qa, "fB", nc.vector)
                rB = pair_sum(qr, "rB", nc.gpsimd)
                fC = pair_sum(ra, "fC", nc.vector)
                rC = pair_sum(rr, "rC", nc.gpsimd)

                # merges in PSUM via fp32r matmuls (512-col halves per bank).
                ps0 = ps.tile([128, _FREE], f32, tag="ps0", name="ps0", bufs=2)
                ps1 = ps.tile([128, _FREE], f32, tag="ps1", name="ps1", bufs=1)
                ps2 = ps.tile([128, _FREE], f32, tag="ps2", name="ps2", bufs=1)

                # group 2's free permute (i,j)->(j,i) rides the moving APs
                fC_s = fC.rearrange("p (a b) -> p a b", a=_D).transpose(
                    [0, 2, 1]
                )
                rC_s = rC.rearrange("p (a b) -> p a b", a=_D)[
                    :, ::-1, ::-1
                ].transpose([0, 2, 1])
                rA_r = rA[:, ::-1]
                rB_r = rB[:, ::-1]

                for h in (0, 1):
                    hs = slice(512 * h, 512 * h + 512)
                    hb = slice(16 * h, 16 * h + 16)
                    nc.tensor.matmul(ps0[:, hs], I12, fA[:, hs],
                                     start=True, stop=False)
                    nc.tensor.matmul(ps0[:, hs], J12, rA_r[:, hs],
                                     start=False, stop=True)
                    nc.tensor.matmul(ps1[:, hs], I12, fB[:, hs],
                                     start=True, stop=False)
                    nc.tensor.matmul(ps1[:, hs], J12, rB_r[:, hs],
                                     start=False, stop=True)
                    nc.tensor.matmul(ps2[:, hs], I12, fC_s[:, hb],
                                     start=True, stop=False)
                    nc.tensor.matmul(ps2[:, hs], J12, rC_s[:, hb],
                                     start=False, stop=True)

                def make_finals(g, ps0, ps1, ps2, cs):
                    def finals():
                        # ps1 holds m1 at (c,j),(k,i); ST -> (c,i),(k,j)
                        T1 = tmp.tile([128, _FREE], f32, tag="T1", name="T1")
                        nc.vector.transpose(T1[:], ps1[:])
                        # ps2 holds sigma(m2) at (c,k),(j,i); ST -> (c,i),(j,k)
                        T2 = tmp.tile([128, _FREE], f32, tag="T2", name="T2")
                        nc.vector.transpose(T2[:], ps2[:])
                        # a = m0 + T2 (in-place into T2)
                        nc.vector.tensor_add(T2[:], T2[:], ps0[:])
                        # o = a + T1 read as (j,k)
                        o = tmp.tile([128, _FREE], f32, tag="o", name="o")
                        o3 = o[:].rearrange("p (a b) -> p a b", a=_D)
                        T2_3 = T2[:].rearrange("p (a b) -> p a b", a=_D)
                        T1_s = T1[:].rearrange(
                            "p (a b) -> p a b", a=_D
                        ).transpose([0, 2, 1])
                        nc.vector.tensor_add(o3, T2_3, T1_s)
                        eng = nc.sync if g % 2 == 0 else nc.scalar
                        eng.dma_start(
                            out=outa[cs].rearrange("c i j k -> (c i) (j k)"),
                            in_=o[:],
                        )
                    return finals

                if finish_prev is not None:
                    finish_prev()
                finish_prev = make_finals(g, ps0, ps1, ps2, cs)

            finish_prev()

    nc.compile()
    return nc


def kernel(ys):
    global _nc
    ys = np.ascontiguousarray(ys, dtype=np.float32)
    assert ys.shape == (_B, _S, _C, _D, _D, _D), ys.shape

    if _nc is None:
        _nc = _build_program()

    from concourse.bass_utils import run_bass_kernel_spmd

    in_maps = []
    for r in range(_NCORE):
        b, q = divmod(r, 4)
        shard = np.ascontiguousarray(ys[b, :, q * _CL:(q + 1) * _CL])
        in_maps.append({"ys": shard})

    res = run_bass_kernel_spmd(_nc, in_maps, list(range(_NCORE)))

    out = np.empty((_B, _C, _D, _D, _D), np.float32)
    for r in range(_NCORE):
        b, q = divmod(r, 4)
        out[b, q * _CL:(q + 1) * _CL] = res.results[r]["out"]

    if res.exec_time_ns is not None:
        print(f"HW exec time: {res.exec_time_ns} ns")
    return out
